# revision 28
# baseline (speedup 1.0000x reference)
"""AtomPoolingLayer Trainium2 kernel (8 NeuronCores, data-parallel over molecules).

Reference computation (per molecule m of 512, atoms n=128, features f=512):
    w = sigmoid(relu(h @ W1 + b1) @ W2 + b2)        # gate, [M, N, 1]
    out[m, f] = sum_n w[m, n] * h[m, n, f]          # weighted pool, [M, F]

Sharding: h split on molecule dim across 8 cores (64 molecules/core); the tiny
MLP weights are replicated. No collectives needed.

Per-core pipeline (bf16 matmuls, DMA roofline ~47us/core for the 16.8MB h):
  SWDGE DMA loads h with f32->bf16 cast inline (halves SBUF writes, no DVE
  cast pass).  h lands as one big [atom, mol, F] bf16 tile.  4-deep software
  pipeline over 4-molecule groups:
      it:   transposes(g)   [PE] -> ht copies (g)  [DVE]
      it+1: stage1(g)=W1.T@hT [PE] -> relu(+b1)    [ACT]
      it+2: stage2(g)=zr.T@W2 [PE] -> sigmoid(+b2) [ACT]
      it+3: stage3(g)=w.T@h   [PE, 4x col-tiled]   -> ACT copy -> DMA out
  The one-iteration gap between transposes and stage1 hides the PE->PSUM->
  DVE->SBUF->PE round trip that otherwise stalls the PE ~620ns twice per
  group.

Single-wait discipline: Matmult / TensorCopy / Activation / DMA dispatch each
support only ONE sync wait (walrus splits a bass matmul into LDWEIGHTS+MATMUL
and puts weights-operand waits on the LDW).  The iteration order makes each
instruction need at most one NEW foreign tick:
  - transposes(g, half0): waits DMA piece g only (the ps_t slot WAR on DVE is
    clock-covered because stage1(g-2) already made PE observe the needed DVE
    tick; ps_t's 3 one-bank slots line up exactly with that coverage).
  - stage1(g-1): waits DVE >= htcopies(g-1) only; its ps_z slot guard is
    covered by the transposes' own PSUM bank-guard self-waits (and by a dummy
    matmul with ht-slice weights in the drain region where no transposes
    remain).
  - mid(g-2): waits ACT >= relu(g-2) only (zr rides its LDWEIGHTS).
  - back(g-3): ACT dep rides the LDW (w_sig weights); MM keeps the ps_o guard.
Probe instructions (cheap ops with explicit sync deps) pre-absorb the DVE/ACT
self-WAW guards each iteration and the outdma completions (two blocks late).
The final drain's tail-sink NOPs (~53ns each on the Sync sequencer) are
emitted mid-loop, ~2 iterations after their semaphores complete, so the
kernel end carries only a handful instead of a ~2us serial NOP train.
A throwaway ident transpose before the loop absorbs the DVE ident-build tick
into PE's clock so transposes(0) need only the DMA wait.
"""

import numpy as np

import concourse.bass as bass
import concourse.mybir as mybir
import concourse.tile as tile
from concourse.bass_utils import run_bass_kernel_spmd

M, N, F = 512, 128, 512
HID = 128
N_CORES = 8
M_PER_CORE = M // N_CORES  # 64
G = 4  # molecules per pipeline group
N_GROUPS = M_PER_CORE // G
FP = mybir.dt.float32
BF = mybir.dt.bfloat16

_AF = mybir.ActivationFunctionType

_LAST_RESULTS = None


def build_bass():
    nc = bass.Bass()

    h_ext = nc.declare_dram_parameter("h", [M_PER_CORE, N, F], FP, isOutput=False)
    w1_ext = nc.declare_dram_parameter("W1", [F, HID], FP, isOutput=False)
    b1_ext = nc.declare_dram_parameter("b1", [HID], FP, isOutput=False)
    w2_ext = nc.declare_dram_parameter("W2", [HID, 1], FP, isOutput=False)
    b2_ext = nc.declare_dram_parameter("b2", [1], FP, isOutput=False)
    out_ext = nc.declare_dram_parameter("out", [M_PER_CORE, F], FP, isOutput=True)

    with tile.TileContext(nc) as tc:
        with (
            tc.tile_pool(name="singles", bufs=1) as singles,
            tc.tile_pool(name="ht", bufs=2) as htp,
            tc.tile_pool(name="zr", bufs=2) as zrp,
            tc.tile_pool(name="ps_t", bufs=3, space="PSUM") as pstp,
            tc.tile_pool(name="ps_z", bufs=2, space="PSUM") as pszp,
            tc.tile_pool(name="ps_w", bufs=1, space="PSUM") as pswp,
            tc.tile_pool(name="ps_o", bufs=2, space="PSUM") as psop,
        ):
            # the full per-core h in bf16: [atom, mol, F] -- 64KB/partition
            hball = singles.tile([128, M_PER_CORE, F], BF)
            h_view = h_ext[:]  # [M_PER_CORE, N, F]

            hdma = [None] * N_GROUPS  # last load piece per group
            group_dmas = [[] for _ in range(N_GROUPS)]
            all_load_dmas = []

            def issue_load(g, split=1):
                # SWDGE cast piece(s) for group g: f32 HBM -> bf16 SBUF with
                # the cast inline in the DMA.  All pieces drain the single SW
                # queue in FIFO order, pipelining back-to-back.  Split pieces
                # let the first/last groups' transposes start half a group
                # earlier (each sub-piece sem is still a single wait for its
                # transpose half).
                per = G // split
                for s in range(split):
                    lo = g * G + s * per
                    dma = nc.gpsimd.dma_start(
                        out=hball[:, lo : lo + per, :],
                        in_=h_view[lo : lo + per].rearrange("g n f -> n g f"),
                    )
                    all_load_dmas.append(dma)
                    group_dmas[g].append(dma)
                hdma[g] = dma
                return dma

            # get bytes flowing before anything else on the gpsimd queue;
            # small first pieces densify early descriptor emission
            issue_load(0, split=2)
            issue_load(1, split=2)

            # ---------------- constants ----------------
            # W1 [F, HID] -> SBUF bf16 [k=128 (F within chunk), c=4 (F chunk), HID]
            # cast happens in the SWDGE DMA itself.
            w1b = singles.tile([128, 4, HID], BF)
            cdma1 = nc.gpsimd.dma_start(
                out=w1b, in_=w1_ext[:].rearrange("(c k) h -> k c h", k=128)
            )

            # b1 [HID] -> [128, 1] f32, absorbed through ACT (its consumer)
            b1raw = singles.tile([128, 1], FP)
            cdma2 = nc.gpsimd.dma_start(
                out=b1raw, in_=b1_ext[:].rearrange("(p o) -> p o", o=1)
            )
            b1s = singles.tile([128, 1], FP)
            nc.scalar.copy(b1s, b1raw)

            # W2 [HID, 1] -> bf16 [128, 1] (cast in DMA), absorbed through ACT
            w2raw = singles.tile([128, 1], BF)
            cdma3 = nc.gpsimd.dma_start(out=w2raw, in_=w2_ext[:])
            w2b = singles.tile([128, 1], BF)
            nc.scalar.copy(w2b, w2raw)

            # b2 [1] broadcast -> [128, 1] f32, absorbed through ACT
            b2raw = singles.tile([128, 1], FP)
            b2_bcast = bass.AP(tensor=b2_ext, offset=0, ap=[[0, 128], [1, 1]])
            cdma4 = nc.gpsimd.dma_start(out=b2raw, in_=b2_bcast)
            b2s = singles.tile([128, 1], FP)
            nc.scalar.copy(b2s, b2raw)

            # identity (f32 gpsimd build, bf16 round on DVE for the transposes)
            ident_f32 = singles.tile([128, 128], FP)
            nc.gpsimd.memset(ident_f32, 0.0)
            ident_mk = nc.gpsimd.affine_select(
                out=ident_f32,
                in_=ident_f32,
                compare_op=mybir.AluOpType.not_equal,
                fill=1.0,
                base=0,
                pattern=[[-1, 128]],
                channel_multiplier=1,
            )
            ident = singles.tile([128, 128], BF)
            nc.vector.tensor_copy(ident, ident_f32)

            # prime one more load piece before the steady-state loop
            issue_load(2)

            # gate weights accumulate here: [atom, molecule] bf16
            # (one spare psum column for the drain-region PE probe matmul)
            w_sig = singles.tile([128, M_PER_CORE], BF)
            psum_w = pswp.tile([128, M_PER_CORE + 1], FP)

            # output staging: molecule j of each group lands on partition 32j
            # (stage-3 col-tiling); one out-DMA per OB_BLOCK groups
            OB_BLOCK = 2
            ob4 = singles.tile([128, 2, OB_BLOCK, F], FP)

            # probe scratch (ACT absorbs outdma completion off the hot path)
            scr_act3 = singles.tile([1, N_GROUPS + 1], FP)
            # probe scratch: disjoint columns, no probe-to-probe deps
            scr_dve = singles.tile([1, N_GROUPS], FP)
            scr_act = singles.tile([128, N_GROUPS + 4], FP)

            # one-time ACT probe past the constant copies
            nc.scalar.copy(scr_act[:, N_GROUPS + 3 : N_GROUPS + 4], b2s)

            from concourse.bass import _add_dep_helper

            chains = {}

            def chained(key, inst):
                prev = chains.get(key)
                if prev is not None:
                    _add_dep_helper(
                        inst.ins, prev.ins, sync=False, reason=f"{key} order"
                    )
                chains[key] = inst
                return inst

            def pe(inst):
                return chained("pe", inst)

            def act(inst):
                return chained("act", inst)

            def dve(inst):
                return chained("dve", inst)

            def probe(chain_key, inst, dep):
                chained(chain_key, inst)
                _add_dep_helper(inst.ins, dep.ins, sync=True, reason="probe")
                return inst

            s3_last = [None] * N_GROUPS
            s1_last = [None] * N_GROUPS
            htcopy_last = [None] * N_GROUPS
            obcopy_last = [None] * N_GROUPS
            outdma = []

            ht_tiles = [None] * N_GROUPS
            zr_tiles = [None] * N_GROUPS

            # throwaway transpose: absorbs the DVE ident tick into PE's clock
            # so the first real transpose needs only its DMA wait.
            ps_warm = pstp.tile([128, 2, 4, 128], BF, tag="ps_t")
            scr_warm = singles.tile([128, 128], BF)
            pe(nc.tensor.transpose(ps_warm[:, 0, 0, :], ident, ident))
            dve(nc.vector.tensor_copy(scr_warm, ps_warm[:, 0, 0, :]))


            def transposes(g, half):
                # PE transposes of molecules (2*half, 2*half+1) of group g
                # into one 1-bank ps_t tile; one DVE copy drains it into ht.
                hb = hball[:, g * G : (g + 1) * G, :]
                if half == 0:
                    if g >= 2:
                        probe(
                            "dve",
                            nc.vector.memset(scr_dve[0:1, g : g + 1], 0.0),
                            htcopy_last[g - 2],
                        )
                    ht_tiles[g] = htp.tile(
                        [128, G, 4, 128], BF, name=f"ht{g}", tag="ht"
                    )
                ht = ht_tiles[g]
                ps_t = pstp.tile([128, 2, 4, 128], BF, tag="ps_t")
                for jj in range(2):
                    j = 2 * half + jj
                    for c in range(4):
                        pe(
                            nc.tensor.transpose(
                                ps_t[:, jj, c, :],
                                hb[:, j, c * 128 : (c + 1) * 128],
                                ident,
                            )
                        )
                htcopy_last[g] = dve(
                    nc.vector.tensor_copy(
                        ht[:, 2 * half : 2 * half + 2, :, :], ps_t
                    )
                )

            def stage1(g):
                # zT = W1.T @ hT (+relu via ACT) for group g
                ht = ht_tiles[g]
                ps_z = pszp.tile([128, G * 128], FP)
                for h_idx in range(2):
                    for c in range(4):
                        s1_last[g] = pe(
                            nc.tensor.matmul(
                                ps_z[:, h_idx * 256 : (h_idx + 1) * 256],
                                w1b[:, c, :],
                                ht[:, 2 * h_idx : 2 * h_idx + 2, c, :],
                                start=(c == 0),
                                stop=(c == 3),
                            )
                        )
                zr = zrp.tile([128, G * 128], BF, name=f"zr{g}", tag="zr")
                zr_tiles[g] = zr
                act(nc.scalar.activation(zr, ps_z, _AF.Relu, bias=b1s))

            def mid_stage(g):
                # stage 2 + sigmoid for group g
                zr = zr_tiles[g]
                for j in range(G):
                    mm = g * G + j
                    pe(
                        nc.tensor.matmul(
                            psum_w[:, mm : mm + 1],
                            zr[:, j * 128 : (j + 1) * 128],
                            w2b,
                            start=True,
                            stop=True,
                        )
                    )
                act(
                    nc.scalar.activation(
                        w_sig[:, g * G : (g + 1) * G],
                        psum_w[:, g * G : (g + 1) * G],
                        _AF.Sigmoid,
                        bias=b2s,
                    )
                )

            def back(g):
                # stage 3 + out staging + block DMA for group g
                hb = hball[:, g * G : (g + 1) * G, :]
                blk = g // OB_BLOCK
                if g % OB_BLOCK == 0 and blk >= 2:
                    # absorb outdma[blk-2]'s completion on ACT (two blocks of
                    # lookahead -- long since complete, so this never stalls)
                    # before its ob half is rewritten; sink it into SP too
                    probe(
                        "act",
                        nc.scalar.mul(
                            scr_act3[0:1, blk : blk + 1],
                            scr_act3[0:1, blk : blk + 1],
                            0.0,
                        ),
                        outdma[blk - 2],
                    )
                    probe(
                        "sp",
                        nc.sync.nop(nofuse=True, hint="tail_sink"),
                        outdma[blk - 2],
                    )
                ps_o4 = psop.tile([128, F], FP)
                for j in range(G):
                    mm = g * G + j
                    s3_last[g] = pe(
                        nc.tensor.matmul(
                            ps_o4[32 * j : 32 * j + 1, :],
                            w_sig[:, mm : mm + 1],
                            hb[:, j, :],
                            start=True,
                            stop=True,
                            tile_position=(0, 32 * j),
                        )
                    )
                obcopy_last[g] = act(
                    nc.scalar.copy(ob4[:, blk % 2, g % OB_BLOCK, :], ps_o4)
                )
                if g % OB_BLOCK == OB_BLOCK - 1:
                    outdma.append(
                        nc.sync.dma_start(
                            out=out_ext[
                                blk * OB_BLOCK * G : (blk + 1) * OB_BLOCK * G
                            ].rearrange("(gi j) f -> j gi f", j=G),
                            in_=ob4[0:128:32, blk % 2, :, :],
                        )
                    )

            # depth-4 software pipeline:
            #   transposes(g) | stage1(g-1) | mid(g-2) | back(g-3)
            act_iter_last = None
            for it in range(N_GROUPS + 3):
                g_t, g_s, g_m, g_b = it, it - 1, it - 2, it - 3
                g_l = g_t + 3
                if g_t < N_GROUPS and 2 < g_l < N_GROUPS:
                    issue_load(g_l, split=2 if g_l >= N_GROUPS - 2 else 1)
                if act_iter_last is not None:
                    probe(
                        "act",
                        nc.scalar.mul(
                            scr_act[0:1, it : it + 1], scr_act[0:1, it : it + 1], 0.0
                        ),
                        act_iter_last,
                    )
                # sink load-piece (and const-DMA) ticks into SP's clock while
                # they are long complete -- keeps the kernel-end drain free of
                # the ~2us serial NOP train it otherwise runs
                if 0 <= it - 2 < N_GROUPS:
                    for dma_done in group_dmas[it - 2]:
                        probe(
                            "sp",
                            nc.sync.nop(nofuse=True, hint="tail_sink"),
                            dma_done,
                        )
                if it == 3:
                    for cd in (cdma1, cdma2, cdma3, cdma4):
                        probe(
                            "sp", nc.sync.nop(nofuse=True, hint="tail_sink"), cd
                        )
                if g_t < N_GROUPS:
                    transposes(g_t, 0)
                if 0 <= g_s < N_GROUPS:
                    if g_t >= N_GROUPS and g_s >= 2:
                        # drain region: no transposes ahead of stage1, so the
                        # ht RAW (DVE) and the ps_z slot wait (PE) would both
                        # land on stage1's first matmul.  Absorb the DVE tick
                        # through a dummy matmul whose WEIGHTS are an ht
                        # slice -- the DVE wait rides its LDWEIGHTS -- so
                        # stage1's matmul keeps only the PE slot wait.
                        pe(
                            nc.tensor.matmul(
                                psum_w[:, M_PER_CORE : M_PER_CORE + 1],
                                ht_tiles[g_s][:, 3, 0, :],
                                ident[:, 0:1],
                                start=True,
                                stop=True,
                            )
                        )
                    stage1(g_s)
                if g_t < N_GROUPS:
                    transposes(g_t, 1)
                if 0 <= g_m < N_GROUPS:
                    mid_stage(g_m)
                if 0 <= g_b < N_GROUPS:
                    back(g_b)
                act_iter_last = chains.get("act")

            # ---- tail: everything except the final out-DMAs and engine
            # chains was already sunk into SP's clock mid-kernel; only a
            # handful of NOPs remain here (the old 38-NOP train cost ~2us
            # of pure kernel-end serialization on the Sync sequencer).
            tail_deps = []
            tail_deps.extend(outdma[-2:])
            tail_deps.append(ident_mk)  # Pool
            tail_deps.append(chains["dve"])  # DVE
            tail_deps.append(chains["act"])  # ACT
            tail_deps.append(s3_last[N_GROUPS - 1])  # PE
            for dep in tail_deps:
                probe("sp", nc.sync.nop(nofuse=True, hint="tail_sink"), dep)

    return nc


_NC_CACHE = None


def kernel(h, W1, b1, W2, b2, _trace=False):
    global _NC_CACHE, _LAST_RESULTS
    h = np.ascontiguousarray(np.asarray(h, dtype=np.float32))
    W1 = np.ascontiguousarray(np.asarray(W1, dtype=np.float32))
    b1 = np.ascontiguousarray(np.asarray(b1, dtype=np.float32))
    W2 = np.ascontiguousarray(np.asarray(W2, dtype=np.float32))
    b2 = np.ascontiguousarray(np.asarray(b2, dtype=np.float32))

    if _NC_CACHE is None:
        _NC_CACHE = build_bass()
    nc = _NC_CACHE

    in_maps = []
    for i in range(N_CORES):
        in_maps.append(
            {
                "h": h[i * M_PER_CORE : (i + 1) * M_PER_CORE],
                "W1": W1,
                "b1": b1,
                "W2": W2,
                "b2": b2,
            }
        )

    res = run_bass_kernel_spmd(
        nc, in_maps, core_ids=list(range(N_CORES)), trace=_trace
    )
    _LAST_RESULTS = res
    out = np.concatenate([np.asarray(r["out"]) for r in res.results], axis=0)
    return out


# revision 31
# speedup vs baseline: 1.0941x; 1.0941x over previous
"""AtomPoolingLayer Trainium2 kernel (8 NeuronCores, data-parallel over molecules).

Reference computation (per molecule m of 512, atoms n=128, features f=512):
    w = sigmoid(relu(h @ W1 + b1) @ W2 + b2)        # gate, [M, N, 1]
    out[m, f] = sum_n w[m, n] * h[m, n, f]          # weighted pool, [M, F]

Sharding: h split on molecule dim across 8 cores (64 molecules/core); the tiny
MLP weights are replicated. No collectives needed.

Per-core pipeline (bf16 matmuls, DMA roofline ~47us/core for the 16.8MB h):
  SWDGE DMA loads h with f32->bf16 cast inline (halves SBUF writes, no DVE
  cast pass).  h lands as one big [atom, mol, F] bf16 tile.  4-deep software
  pipeline over 4-molecule groups:
      it:   transposes(g)   [PE] -> ht copies (g)  [DVE]
      it+1: stage1(g)=W1.T@hT [PE] -> relu(+b1)    [ACT]
      it+2: stage2(g)=zr.T@W2 [PE] -> sigmoid(+b2) [ACT]
      it+3: stage3(g)=w.T@h   [PE, 4x col-tiled]   -> ACT copy -> DMA out
  The one-iteration gap between transposes and stage1 hides the PE->PSUM->
  DVE->SBUF->PE round trip that otherwise stalls the PE ~620ns twice per
  group.

Single-wait discipline: Matmult / TensorCopy / Activation / DMA dispatch each
support only ONE sync wait (walrus splits a bass matmul into LDWEIGHTS+MATMUL
and puts weights-operand waits on the LDW).  The iteration order makes each
instruction need at most one NEW foreign tick:
  - transposes(g, half0): waits DMA piece g only (the ps_t slot WAR on DVE is
    clock-covered because stage1(g-2) already made PE observe the needed DVE
    tick; ps_t's 3 one-bank slots line up exactly with that coverage).
  - stage1(g-1): waits DVE >= htcopies(g-1) only; its ps_z slot guard is
    covered by the transposes' own PSUM bank-guard self-waits (and by a dummy
    matmul with ht-slice weights in the drain region where no transposes
    remain).
  - mid(g-2): waits ACT >= relu(g-2) only (zr rides its LDWEIGHTS).
  - back(g-3): ACT dep rides the LDW (w_sig weights); MM keeps the ps_o guard.
Probe instructions (cheap ops with explicit sync deps) pre-absorb the DVE/ACT
self-WAW guards each iteration and the outdma completions (two blocks late).
The final drain's tail-sink NOPs (~53ns each on the Sync sequencer) are
emitted mid-loop, ~2 iterations after their semaphores complete, so the
kernel end carries only a handful instead of a ~2us serial NOP train.
A throwaway ident transpose before the loop absorbs the DVE ident-build tick
into PE's clock so transposes(0) need only the DMA wait.
"""

import numpy as np

import concourse.bass as bass
import concourse.mybir as mybir
import concourse.tile as tile
from concourse.bass_utils import run_bass_kernel_spmd

M, N, F = 512, 128, 512
HID = 128
N_CORES = 8
M_PER_CORE = M // N_CORES  # 64
G = 4  # molecules per pipeline group
N_GROUPS = M_PER_CORE // G
FP = mybir.dt.float32
BF = mybir.dt.bfloat16

_AF = mybir.ActivationFunctionType

_LAST_RESULTS = None


def build_bass():
    nc = bass.Bass()

    h_ext = nc.declare_dram_parameter("h", [M_PER_CORE, N, F], FP, isOutput=False)
    w1_ext = nc.declare_dram_parameter("W1", [F, HID], FP, isOutput=False)
    b1_ext = nc.declare_dram_parameter("b1", [HID], FP, isOutput=False)
    w2_ext = nc.declare_dram_parameter("W2", [HID, 1], FP, isOutput=False)
    b2_ext = nc.declare_dram_parameter("b2", [1], FP, isOutput=False)
    out_ext = nc.declare_dram_parameter("out", [M_PER_CORE, F], FP, isOutput=True)

    with tile.TileContext(nc) as tc:
        with (
            tc.tile_pool(name="singles", bufs=1) as singles,
            tc.tile_pool(name="ht", bufs=2) as htp,
            tc.tile_pool(name="zr", bufs=2) as zrp,
            tc.tile_pool(name="ps_t", bufs=3, space="PSUM") as pstp,
            tc.tile_pool(name="ps_z", bufs=2, space="PSUM") as pszp,
            tc.tile_pool(name="ps_w", bufs=1, space="PSUM") as pswp,
            tc.tile_pool(name="ps_o", bufs=2, space="PSUM") as psop,
        ):
            # the full per-core h in bf16: [atom, mol, F] -- 64KB/partition
            hball = singles.tile([128, M_PER_CORE, F], BF)
            h_view = h_ext[:]  # [M_PER_CORE, N, F]

            hdma = [None] * N_GROUPS  # last load piece per group
            group_dmas = [[] for _ in range(N_GROUPS)]
            all_load_dmas = []

            def issue_load(g, split=1):
                # SWDGE cast piece(s) for group g: f32 HBM -> bf16 SBUF with
                # the cast inline in the DMA.  All pieces drain the single SW
                # queue in FIFO order, pipelining back-to-back.  Split pieces
                # let the first/last groups' transposes start half a group
                # earlier (each sub-piece sem is still a single wait for its
                # transpose half).
                per = G // split
                for s in range(split):
                    lo = g * G + s * per
                    dma = nc.gpsimd.dma_start(
                        out=hball[:, lo : lo + per, :],
                        in_=h_view[lo : lo + per].rearrange("g n f -> n g f"),
                    )
                    all_load_dmas.append(dma)
                    group_dmas[g].append(dma)
                hdma[g] = dma
                return dma

            # get bytes flowing before anything else on the gpsimd queue;
            # small first pieces densify early descriptor emission
            issue_load(0, split=2)
            issue_load(1, split=2)

            # ---------------- constants ----------------
            # W1 [F, HID] -> SBUF bf16 [k=128 (F within chunk), c=4 (F chunk), HID]
            # cast happens in the SWDGE DMA itself.
            w1b = singles.tile([128, 4, HID], BF)
            cdma1 = nc.gpsimd.dma_start(
                out=w1b, in_=w1_ext[:].rearrange("(c k) h -> k c h", k=128)
            )

            # b1 [HID] -> [128, 1] f32, absorbed through ACT (its consumer)
            b1raw = singles.tile([128, 1], FP)
            cdma2 = nc.gpsimd.dma_start(
                out=b1raw, in_=b1_ext[:].rearrange("(p o) -> p o", o=1)
            )
            b1s = singles.tile([128, 1], FP)
            nc.scalar.copy(b1s, b1raw)

            # W2 [HID, 1] -> bf16 [128, 1] (cast in DMA), absorbed through ACT
            w2raw = singles.tile([128, 1], BF)
            cdma3 = nc.gpsimd.dma_start(out=w2raw, in_=w2_ext[:])
            w2b = singles.tile([128, 1], BF)
            nc.scalar.copy(w2b, w2raw)

            # b2 [1] broadcast -> [128, 1] f32, absorbed through ACT
            b2raw = singles.tile([128, 1], FP)
            b2_bcast = bass.AP(tensor=b2_ext, offset=0, ap=[[0, 128], [1, 1]])
            cdma4 = nc.gpsimd.dma_start(out=b2raw, in_=b2_bcast)
            b2s = singles.tile([128, 1], FP)
            nc.scalar.copy(b2s, b2raw)

            # identity (f32 gpsimd build, bf16 round on DVE for the transposes)
            ident_f32 = singles.tile([128, 128], FP)
            nc.gpsimd.memset(ident_f32, 0.0)
            ident_mk = nc.gpsimd.affine_select(
                out=ident_f32,
                in_=ident_f32,
                compare_op=mybir.AluOpType.not_equal,
                fill=1.0,
                base=0,
                pattern=[[-1, 128]],
                channel_multiplier=1,
            )
            ident = singles.tile([128, 128], BF)
            nc.vector.tensor_copy(ident, ident_f32)

            # prime one more load piece before the steady-state loop
            issue_load(2)

            # gate weights accumulate here: [atom, molecule] bf16
            # (one spare psum column for the drain-region PE probe matmul)
            w_sig = singles.tile([128, M_PER_CORE], BF)
            psum_w = pswp.tile([128, M_PER_CORE + 2], FP)

            # output staging: molecule j of each group lands on partition 32j
            # (stage-3 col-tiling); one out-DMA per OB_BLOCK groups
            OB_BLOCK = 2
            ob4 = singles.tile([128, 2, OB_BLOCK, F], FP)

            # probe scratch (ACT absorbs outdma completion off the hot path)
            scr_act3 = singles.tile([1, N_GROUPS + 1], FP)
            # probe scratch: disjoint columns, no probe-to-probe deps
            scr_dve = singles.tile([1, N_GROUPS], FP)
            scr_act = singles.tile([128, N_GROUPS + 4], FP)

            # one-time ACT probe past the constant copies
            nc.scalar.copy(scr_act[:, N_GROUPS + 3 : N_GROUPS + 4], b2s)

            from concourse.bass import _add_dep_helper

            chains = {}

            def chained(key, inst):
                prev = chains.get(key)
                if prev is not None:
                    _add_dep_helper(
                        inst.ins, prev.ins, sync=False, reason=f"{key} order"
                    )
                chains[key] = inst
                return inst

            def pe(inst):
                return chained("pe", inst)

            def act(inst):
                return chained("act", inst)

            def dve(inst):
                return chained("dve", inst)

            def probe(chain_key, inst, dep):
                chained(chain_key, inst)
                _add_dep_helper(inst.ins, dep.ins, sync=True, reason="probe")
                return inst

            s3_last = [None] * N_GROUPS
            s1_last = [None] * N_GROUPS
            htcopy_last = [None] * N_GROUPS
            obcopy_last = [None] * N_GROUPS
            outdma = []

            ht_tiles = [None] * N_GROUPS
            zr_tiles = [None] * N_GROUPS

            # throwaway transpose: absorbs the DVE ident tick into PE's clock
            # so the first real transpose needs only its DMA wait.
            ps_warm = pstp.tile([128, 2, 4, 128], BF, tag="ps_t")
            scr_warm = singles.tile([128, 128], BF)
            pe(nc.tensor.transpose(ps_warm[:, 0, 0, :], ident, ident))
            dve(nc.vector.tensor_copy(scr_warm, ps_warm[:, 0, 0, :]))


            def transposes(g, half):
                # PE transposes of molecules (2*half, 2*half+1) of group g
                # into one 1-bank ps_t tile; one DVE copy drains it into ht.
                hb = hball[:, g * G : (g + 1) * G, :]
                if half == 0:
                    if g >= 2:
                        probe(
                            "dve",
                            nc.vector.memset(scr_dve[0:1, g : g + 1], 0.0),
                            htcopy_last[g - 2],
                        )
                    ht_tiles[g] = htp.tile(
                        [128, G, 4, 128], BF, name=f"ht{g}", tag="ht"
                    )
                ht = ht_tiles[g]
                ps_t = pstp.tile([128, 2, 4, 128], BF, tag="ps_t")
                for jj in range(2):
                    j = 2 * half + jj
                    for c in range(4):
                        pe(
                            nc.tensor.transpose(
                                ps_t[:, jj, c, :],
                                hb[:, j, c * 128 : (c + 1) * 128],
                                ident,
                            )
                        )
                htcopy_last[g] = dve(
                    nc.vector.tensor_copy(
                        ht[:, 2 * half : 2 * half + 2, :, :], ps_t
                    )
                )

            def stage1_half(g, h_idx, ps_z):
                ht = ht_tiles[g]
                for c in range(4):
                    s1_last[g] = pe(
                        nc.tensor.matmul(
                            ps_z[:, h_idx * 256 : (h_idx + 1) * 256],
                            w1b[:, c, :],
                            ht[:, 2 * h_idx : 2 * h_idx + 2, c, :],
                            start=(c == 0),
                            stop=(c == 3),
                        )
                    )

            def stage1(g):
                # zT = W1.T @ hT (+relu via ACT) for group g
                ps_z = pszp.tile([128, G * 128], FP)
                stage1_half(g, 0, ps_z)
                stage1_half(g, 1, ps_z)
                zr = zrp.tile([128, G * 128], BF, name=f"zr{g}", tag="zr")
                zr_tiles[g] = zr
                act(nc.scalar.activation(zr, ps_z, _AF.Relu, bias=b1s))

            def mid_stage(g):
                # stage 2 + sigmoid for group g
                zr = zr_tiles[g]
                for j in range(G):
                    mm = g * G + j
                    pe(
                        nc.tensor.matmul(
                            psum_w[:, mm : mm + 1],
                            zr[:, j * 128 : (j + 1) * 128],
                            w2b,
                            start=True,
                            stop=True,
                        )
                    )
                act(
                    nc.scalar.activation(
                        w_sig[:, g * G : (g + 1) * G],
                        psum_w[:, g * G : (g + 1) * G],
                        _AF.Sigmoid,
                        bias=b2s,
                    )
                )

            def back(g):
                # stage 3 + out staging + block DMA for group g
                hb = hball[:, g * G : (g + 1) * G, :]
                blk = g // OB_BLOCK
                if g % OB_BLOCK == 0 and blk >= 2:
                    # absorb outdma[blk-2]'s completion on ACT (two blocks of
                    # lookahead -- long since complete, so this never stalls)
                    # before its ob half is rewritten; sink it into SP too
                    probe(
                        "act",
                        nc.scalar.mul(
                            scr_act3[0:1, blk : blk + 1],
                            scr_act3[0:1, blk : blk + 1],
                            0.0,
                        ),
                        outdma[blk - 2],
                    )
                    probe(
                        "sp",
                        nc.sync.nop(nofuse=True, hint="tail_sink"),
                        outdma[blk - 2],
                    )
                ps_o4 = psop.tile([128, F], FP)
                for j in range(G):
                    mm = g * G + j
                    s3_last[g] = pe(
                        nc.tensor.matmul(
                            ps_o4[32 * j : 32 * j + 1, :],
                            w_sig[:, mm : mm + 1],
                            hb[:, j, :],
                            start=True,
                            stop=True,
                            tile_position=(0, 32 * j),
                        )
                    )
                obcopy_last[g] = act(
                    nc.scalar.copy(ob4[:, blk % 2, g % OB_BLOCK, :], ps_o4)
                )
                if g % OB_BLOCK == OB_BLOCK - 1:
                    outdma.append(
                        nc.sync.dma_start(
                            out=out_ext[
                                blk * OB_BLOCK * G : (blk + 1) * OB_BLOCK * G
                            ].rearrange("(gi j) f -> j gi f", j=G),
                            in_=ob4[0:128:32, blk % 2, :, :],
                        )
                    )

            # depth-4 software pipeline:
            #   transposes(g) | stage1(g-1) | mid(g-2) | back(g-3)
            act_iter_last = None
            for it in range(N_GROUPS + 3):
                g_t, g_s, g_m, g_b = it, it - 1, it - 2, it - 3
                g_l = g_t + 3
                if g_t < N_GROUPS and 2 < g_l < N_GROUPS:
                    issue_load(g_l, split=2 if g_l >= N_GROUPS - 2 else 1)
                if act_iter_last is not None:
                    probe(
                        "act",
                        nc.scalar.mul(
                            scr_act[0:1, it : it + 1], scr_act[0:1, it : it + 1], 0.0
                        ),
                        act_iter_last,
                    )
                # sink load-piece (and const-DMA) ticks into SP's clock while
                # they are long complete -- keeps the kernel-end drain free of
                # the ~2us serial NOP train it otherwise runs
                if 0 <= it - 2 < N_GROUPS:
                    for dma_done in group_dmas[it - 2]:
                        probe(
                            "sp",
                            nc.sync.nop(nofuse=True, hint="tail_sink"),
                            dma_done,
                        )
                if it == 3:
                    for cd in (cdma1, cdma2, cdma3, cdma4):
                        probe(
                            "sp", nc.sync.nop(nofuse=True, hint="tail_sink"), cd
                        )
                if g_t < N_GROUPS:
                    transposes(g_t, 0)
                drain_split = g_t >= N_GROUPS and g_s == N_GROUPS - 1
                if 0 <= g_s < N_GROUPS:
                    if g_t >= N_GROUPS:
                        # drain region: no transposes ahead of stage1, so the
                        # ht RAW (DVE) and the ps_z slot wait (PE) would both
                        # land on stage1's first matmul.  Absorb the DVE tick
                        # through a dummy matmul whose WEIGHTS are an ht
                        # slice -- the DVE wait rides its LDWEIGHTS -- so
                        # stage1's matmul keeps only the PE slot wait.
                        pe(
                            nc.tensor.matmul(
                                psum_w[:, M_PER_CORE : M_PER_CORE + 1],
                                ht_tiles[g_s][:, 1 if drain_split else 3, 0, :],
                                ident[:, 0:1],
                                start=True,
                                stop=True,
                            )
                        )
                    if drain_split:
                        # last group: run stage1+relu in halves so the chain
                        # behind the final load piece only carries half a
                        # group of stage1/relu latency
                        ps_z_l = pszp.tile([128, G * 128], FP, tag="ps_z")
                        zr_l = zrp.tile([128, G * 128], BF, name="zr_l", tag="zr")
                        zr_tiles[g_s] = zr_l
                        stage1_half(g_s, 0, ps_z_l)
                        act(
                            nc.scalar.activation(
                                zr_l[:, 0:256], ps_z_l[:, 0:256], _AF.Relu,
                                bias=b1s,
                            )
                        )
                    else:
                        stage1(g_s)
                if g_t < N_GROUPS:
                    transposes(g_t, 1)
                if 0 <= g_m < N_GROUPS:
                    mid_stage(g_m)
                if 0 <= g_b < N_GROUPS:
                    back(g_b)
                if drain_split:
                    # second half: dummy absorbs htcopy(15, half1)'s DVE tick
                    pe(
                        nc.tensor.matmul(
                            psum_w[:, M_PER_CORE + 1 : M_PER_CORE + 2],
                            ht_tiles[g_s][:, 3, 0, :],
                            ident[:, 0:1],
                            start=True,
                            stop=True,
                        )
                    )
                    stage1_half(g_s, 1, ps_z_l)
                    # absorb the pending ACT self tick so relu-h1 keeps a
                    # single (PE) wait
                    probe(
                        "act",
                        nc.scalar.mul(
                            scr_act[0:1, 0:1], scr_act[0:1, 0:1], 0.0
                        ),
                        chains["act"],
                    )
                    act(
                        nc.scalar.activation(
                            zr_l[:, 256:512], ps_z_l[:, 256:512], _AF.Relu,
                            bias=b1s,
                        )
                    )
                act_iter_last = chains.get("act")

            # ---- tail: everything except the final out-DMAs and engine
            # chains was already sunk into SP's clock mid-kernel; only a
            # handful of NOPs remain here (the old 38-NOP train cost ~2us
            # of pure kernel-end serialization on the Sync sequencer).
            tail_deps = []
            tail_deps.extend(outdma[-2:])
            tail_deps.append(ident_mk)  # Pool
            tail_deps.append(chains["dve"])  # DVE
            tail_deps.append(chains["act"])  # ACT
            tail_deps.append(s3_last[N_GROUPS - 1])  # PE
            for dep in tail_deps:
                probe("sp", nc.sync.nop(nofuse=True, hint="tail_sink"), dep)

    return nc


_NC_CACHE = None


def kernel(h, W1, b1, W2, b2, _trace=False):
    global _NC_CACHE, _LAST_RESULTS
    h = np.ascontiguousarray(np.asarray(h, dtype=np.float32))
    W1 = np.ascontiguousarray(np.asarray(W1, dtype=np.float32))
    b1 = np.ascontiguousarray(np.asarray(b1, dtype=np.float32))
    W2 = np.ascontiguousarray(np.asarray(W2, dtype=np.float32))
    b2 = np.ascontiguousarray(np.asarray(b2, dtype=np.float32))

    if _NC_CACHE is None:
        _NC_CACHE = build_bass()
    nc = _NC_CACHE

    in_maps = []
    for i in range(N_CORES):
        in_maps.append(
            {
                "h": h[i * M_PER_CORE : (i + 1) * M_PER_CORE],
                "W1": W1,
                "b1": b1,
                "W2": W2,
                "b2": b2,
            }
        )

    res = run_bass_kernel_spmd(
        nc, in_maps, core_ids=list(range(N_CORES)), trace=_trace
    )
    _LAST_RESULTS = res
    out = np.concatenate([np.asarray(r["out"]) for r in res.results], axis=0)
    return out
